# revision 4
# baseline (speedup 1.0000x reference)
"""Trainium2 Bass kernel for nn_DownBlock (PacConv1x1 -> PReLU -> Conv6x6s2 -> PReLU).

Math notes:
  - The PacConv2d adaptive kernel is exp(-0.5*||g-g||^2) == 1 exactly, so the
    guide tensor is mathematically unused: stage 1 is a plain 1x1 conv.
  - Stage 1: h[f,y,x] = prelu(sum_c pac_w[f,c] * x[c,y,x] + pac_b[f], alpha1)
  - Stage 2: 6x6 stride-2 conv with padding 2, + bias, prelu.

Implementation (per core, 2 of the 16 batch images, data-parallel over batch):
  - Stage 1 is a K=128 matmul per 512-position tile; the rhs access pattern
    picks x-parity phases so the epilogue (one fused Prelu ACT op) writes
    directly into the stage-2 input layout:
        Hx[(px, f), y+2, x//2 + 1]   (128 partitions, 132x66 image, zero halo)
  - Stage 2 (stride-2 6x6 conv) contracts (px, c) = 128 partitions per tap:
        out[o, i, j] = sum_{ky, n} Wp[ky,n][(px,c), o].T @ Hx[:, 2i+ky, j+n]
    = 18 accumulating K=128/M=64/N=512 matmuls per 8-row output block,
    then one fused Prelu ACT epilogue and a DMA out.
  - All matmul operands are float32r (TF32-class, full-rate on the PE).
"""
import numpy as np

import concourse.bacc as bacc
import concourse.mybir as mybir
from concourse.tile import TileContext
from concourse.bass_utils import run_bass_kernel_spmd
from concourse.masks import make_identity

F32 = mybir.dt.float32
F32R = mybir.dt.float32r

N_CORES = 8
B_TOTAL = 16
B_PER_CORE = B_TOTAL // N_CORES  # 2
CIN = 128
CG = 64  # guide channels (unused)
F = 64   # intermediate / output channels
H = W = 128
HO = WO = 64
K = 6
# phase image: rows 0..131 (y+2), cols 0..65 (x//2+1), zero halo
PR = 132
PC = 66

_CACHE = {}


def _build():
    nc = bacc.Bacc("TRN2", target_bir_lowering=False, debug=False)

    x = nc.declare_dram_parameter("x", [B_PER_CORE, CIN, H, W], F32, isOutput=False)
    pac_w = nc.declare_dram_parameter("pac_w", [F, CIN], F32, isOutput=False)
    pac_b = nc.declare_dram_parameter("pac_b", [F], F32, isOutput=False)
    alpha1 = nc.declare_dram_parameter("alpha1", [1], F32, isOutput=False)
    conv_w = nc.declare_dram_parameter("conv_w", [F, F * K * K], F32, isOutput=False)
    conv_b = nc.declare_dram_parameter("conv_b", [F], F32, isOutput=False)
    alpha2 = nc.declare_dram_parameter("alpha2", [1], F32, isOutput=False)
    out = nc.declare_dram_parameter("out", [B_PER_CORE, F, HO, WO], F32, isOutput=True)

    PRELU = mybir.ActivationFunctionType.Prelu
    COPY = mybir.ActivationFunctionType.Copy

    with TileContext(nc) as tc:
        with (
            tc.tile_pool(name="const", bufs=1) as const,
            tc.tile_pool(name="xin", bufs=3) as xin,
            tc.tile_pool(name="hx", bufs=1) as hxp,
            tc.tile_pool(name="ob", bufs=3) as obp,
            tc.tile_pool(name="psA", bufs=3, space="PSUM") as psA,
            tc.tile_pool(name="psW", bufs=2, space="PSUM") as psW,
        ):
            # ---------------- constants / weight prep ----------------
            ident_f = const.tile([F, F], F32)
            make_identity(nc, ident_f[:])
            ident = const.tile([F, F], F32R)
            nc.vector.tensor_copy(ident[:], ident_f[:])

            # per-partition scalars (read against psum partitions 0:64)
            b1 = const.tile([F, 1], F32)
            b2 = const.tile([F, 1], F32)
            a1 = const.tile([F, 1], F32)
            a2 = const.tile([F, 1], F32)
            nc.sync.dma_start(out=b1[:], in_=pac_b[:, None])
            nc.sync.dma_start(out=b2[:], in_=conv_b[:, None])
            nc.sync.dma_start(out=a1[:], in_=alpha1.broadcast_to([F, 1]))
            nc.sync.dma_start(out=a2[:], in_=alpha2.broadcast_to([F, 1]))

            # staging of raw weights: [F(part), Cin] and [F(part), F*36]
            pac_stage = const.tile([F, CIN], F32R)
            w_stage = const.tile([F, F * K * K], F32R)
            nc.sync.dma_start(out=pac_stage[:], in_=pac_w[:].bitcast(F32R))
            nc.sync.dma_start(out=w_stage[:], in_=conv_w[:].bitcast(F32R))

            # pac_wT[c, f] = pac_w[f, c]: two 64-col transposes via matmul w/ identity
            pac_wT = const.tile([CIN, F], F32R)
            for half in range(2):
                pt = psW.tile([F, F], F32, tag="wprep")
                nc.tensor.matmul(
                    pt[:], pac_stage[:, half * 64:(half + 1) * 64], ident[:],
                    start=True, stop=True,
                )
                nc.scalar.activation(pac_wT[half * 64:(half + 1) * 64, :], pt[:], COPY)

            # Wp[t18 = ky*3+n][(px, c), o] = conv_w[o, c, ky, 2n+px]
            wp = const.tile([CIN, 18 * F], F32R)
            for ky in range(K):
                for n in range(3):
                    t18 = ky * 3 + n
                    for px in range(2):
                        pt = psW.tile([F, F], F32, tag="wprep")
                        # lhsT: [o(64 part), c(64)] strided pick of tap (ky, 2n+px)
                        lhsT = w_stage[:, ky * K + 2 * n + px::K * K]
                        nc.tensor.matmul(pt[:], lhsT, ident[:], start=True, stop=True)
                        nc.scalar.activation(
                            wp[px * 64:(px + 1) * 64, t18 * F:(t18 + 1) * F], pt[:], COPY
                        )

            # ---------------- phase tensors + halo zeroing ----------------
            zrow = const.tile([CIN, 2 * PC], F32)
            nc.gpsimd.memset(zrow[:], 0.0)

            hx = [
                hxp.tile([CIN, PR * PC], F32R, tag=f"hx{b}", name=f"hx{b}")
                for b in range(B_PER_CORE)
            ]
            for b in range(B_PER_CORE):
                t = hx[b]
                # top rows 0..1, bottom rows 130..131
                nc.scalar.activation(t[:, 0:2 * PC], zrow[:], COPY)
                nc.scalar.activation(t[:, 130 * PC:132 * PC], zrow[:], COPY)
                # left col 0 and right col 65 stripes (132 rows each)
                nc.scalar.activation(t[:, 0:PR * PC:PC], zrow[:, 0:PR], COPY)
                nc.scalar.activation(t[:, 65:PR * PC:PC], zrow[:, 0:PR], COPY)

            # ---------------- main pipeline ----------------
            for b in range(B_PER_CORE):
                hxb = hx[b].rearrange("p (r c) -> p r c", c=PC)
                for t in range(8):  # 16-row x chunks
                    xt = xin.tile([CIN, 16 * W], F32R, tag="xt")
                    xtv = xt[:].rearrange("p (r c) -> p r c", r=16)
                    nc.sync.dma_start(
                        out=xtv,
                        in_=x[b, :, 16 * t:16 * t + 16, :].bitcast(F32R),
                    )
                    for h3 in range(2):  # 8-row halves
                        for px in range(2):  # x parity
                            ps = psA.tile([F, 8, 64], F32, tag="s1")
                            nc.tensor.matmul(
                                ps[:],
                                pac_wT[:],
                                xtv[:, 8 * h3:8 * h3 + 8, px::2],
                                start=True, stop=True,
                            )
                            # fused bias+prelu, writing the phase-image layout
                            r0 = 16 * t + 8 * h3 + 2
                            nc.scalar.activation(
                                hxb[px * 64:(px + 1) * 64, r0:r0 + 8, 1:65],
                                ps[:], PRELU, bias=b1[:], scale=1.0, alpha=a1[:],
                            )
                    if t >= 1:
                        _stage2_block(nc, tc, psA, obp, hxb, wp, b2, a2, out, b, t - 1)
                _stage2_block(nc, tc, psA, obp, hxb, wp, b2, a2, out, b, 7)

    nc.compile()
    return nc


def _stage2_block(nc, tc, psA, obp, hxb, wp, b2, a2, out, b, ib):
    """18 accumulating taps -> prelu -> dma for output rows [8*ib, 8*ib+8)."""
    PRELU = mybir.ActivationFunctionType.Prelu
    ps = psA.tile([F, 8, 64], F32, tag="s2")
    for ky in range(K):
        for n in range(3):
            t18 = ky * 3 + n
            r0 = 16 * ib + ky
            rhs = hxb[:, r0:min(r0 + 16, PR):2, n:n + 64]
            nc.tensor.matmul(
                ps[:], wp[:, t18 * F:(t18 + 1) * F], rhs,
                start=(t18 == 0), stop=(t18 == 17),
            )
    ot = obp.tile([F, 8, 64], F32, tag="ot")
    nc.scalar.activation(ot[:], ps[:], PRELU, bias=b2[:], scale=1.0, alpha=a2[:])
    nc.sync.dma_start(out=out[b, :, 8 * ib:8 * ib + 8, :], in_=ot[:])


def _get_nc():
    if "nc" not in _CACHE:
        _CACHE["nc"] = _build()
    return _CACHE["nc"]


def kernel(x, guide, pac_w, pac_b, alpha1, alpha2, conv_w, conv_b, **_unused):
    # guide is mathematically unused (adaptive kernel == exp(0) == 1)
    del guide
    x = np.ascontiguousarray(x, dtype=np.float32)
    shared = {
        "pac_w": np.ascontiguousarray(pac_w, dtype=np.float32).reshape(F, CIN),
        "pac_b": np.ascontiguousarray(pac_b, dtype=np.float32),
        "alpha1": np.ascontiguousarray(alpha1, dtype=np.float32),
        "conv_w": np.ascontiguousarray(conv_w, dtype=np.float32).reshape(F, F * K * K),
        "conv_b": np.ascontiguousarray(conv_b, dtype=np.float32),
        "alpha2": np.ascontiguousarray(alpha2, dtype=np.float32),
    }
    in_maps = [
        {"x": np.ascontiguousarray(x[i * B_PER_CORE:(i + 1) * B_PER_CORE]), **shared}
        for i in range(N_CORES)
    ]
    nc = _get_nc()
    res = run_bass_kernel_spmd(
        nc, in_maps, list(range(N_CORES)), trace=_CACHE.get("trace", False)
    )
    _CACHE["last_result"] = res
    return np.concatenate([r["out"] for r in res.results], axis=0)


# revision 8
# speedup vs baseline: 8.3294x; 8.3294x over previous
"""Trainium2 Bass kernel for nn_DownBlock (PacConv1x1 -> PReLU -> Conv6x6s2 -> PReLU).

Math notes:
  - The PacConv2d adaptive kernel is exp(-0.5*||g-g||^2) == 1 exactly, so the
    guide tensor is mathematically unused: stage 1 is a plain 1x1 conv.
  - Stage 1: h[f,y,x] = prelu(sum_c pac_w[f,c] * x[c,y,x] + pac_b[f], alpha1)
  - Stage 2: 6x6 stride-2 conv with padding 2, + bias, prelu.

Implementation (per core, 2 of the 16 batch images, data-parallel over batch):
  - Stage 1 is a K=128 matmul per 512-position tile; the rhs access pattern
    picks x-parity phases so the epilogue (one fused Prelu ACT op) writes
    directly into the stage-2 input layout:
        Hx[(px, f), y+2, x//2 + 1]   (128 partitions, 132x66 image, zero halo)
  - Stage 2 (stride-2 6x6 conv) contracts (px, c) = 128 partitions per tap:
        out[o, i, j] = sum_{ky, n} Wp[ky,n][(px,c), o].T @ Hx[:, 2i+ky, j+n]
    = 18 accumulating K=128/M=64/N=512 matmuls per 8-row output block,
    then one fused Prelu ACT epilogue and a DMA out.
  - All matmul operands are float32r (TF32-class, full-rate on the PE).
"""
import numpy as np

import concourse.bacc as bacc
import concourse.mybir as mybir
from concourse.tile import TileContext
from concourse.bass_utils import run_bass_kernel_spmd
from concourse.masks import make_identity

F32 = mybir.dt.float32
F32R = mybir.dt.float32r

N_CORES = 8
B_TOTAL = 16
B_PER_CORE = B_TOTAL // N_CORES  # 2
CIN = 128
CG = 64  # guide channels (unused)
F = 64   # intermediate / output channels
H = W = 128
HO = WO = 64
K = 6
# phase image: rows 0..131 (y+2), cols 0..65 (x//2+1), zero halo
PR = 132
PC = 66

_CACHE = {}


def _build(repeat=1):
    """Build the Bass module.  repeat>1 re-emits the main pipeline that many
    times back-to-back (bench-only: lets wall-clock slope isolate per-pass
    device time from the ~0.5ms axon dispatch overhead)."""
    nc = bacc.Bacc("TRN2", target_bir_lowering=False, debug=False)

    x = nc.declare_dram_parameter("x", [B_PER_CORE, CIN, H, W], F32, isOutput=False)
    pac_w = nc.declare_dram_parameter("pac_w", [F, CIN], F32, isOutput=False)
    pac_b = nc.declare_dram_parameter("pac_b", [F], F32, isOutput=False)
    alpha1 = nc.declare_dram_parameter("alpha1", [1], F32, isOutput=False)
    conv_w = nc.declare_dram_parameter("conv_w", [F, F * K * K], F32, isOutput=False)
    conv_b = nc.declare_dram_parameter("conv_b", [F], F32, isOutput=False)
    alpha2 = nc.declare_dram_parameter("alpha2", [1], F32, isOutput=False)
    out = nc.declare_dram_parameter("out", [B_PER_CORE, F, HO, WO], F32, isOutput=True)

    PRELU = mybir.ActivationFunctionType.Prelu
    COPY = mybir.ActivationFunctionType.Copy

    with TileContext(nc) as tc:
        with (
            tc.tile_pool(name="const", bufs=1) as const,
            tc.tile_pool(name="xin", bufs=6) as xin,
            tc.tile_pool(name="hx", bufs=1) as hxp,
            tc.tile_pool(name="ob", bufs=3) as obp,
            tc.tile_pool(name="psA", bufs=4, space="PSUM") as psA,
        ):
            # ---------------- constants / weight prep ----------------
            ident_f = const.tile([F, F], F32)
            make_identity(nc, ident_f[:])
            ident = const.tile([F, F], F32R)
            nc.vector.tensor_copy(ident[:], ident_f[:])

            # per-partition scalars (read against psum partitions 0:64)
            b1 = const.tile([F, 1], F32)
            b2 = const.tile([F, 1], F32)
            a1 = const.tile([F, 1], F32)
            a2 = const.tile([F, 1], F32)
            nc.sync.dma_start(out=b1[:], in_=pac_b[:, None])
            nc.sync.dma_start(out=b2[:], in_=conv_b[:, None])
            nc.sync.dma_start(out=a1[:], in_=alpha1.broadcast_to([F, 1]))
            nc.sync.dma_start(out=a2[:], in_=alpha2.broadcast_to([F, 1]))

            # staging of raw weights: [F(part), Cin] and [F(part), F*36]
            pac_stage = const.tile([F, CIN], F32R)
            w_stage = const.tile([F, F * K * K], F32R)
            nc.sync.dma_start(out=pac_stage[:], in_=pac_w[:].bitcast(F32R))
            nc.sync.dma_start(out=w_stage[:], in_=conv_w[:].bitcast(F32R))

            # pac_wT[c, f] = pac_w[f, c]: two 64-col transposes via matmul w/ identity
            pac_wT = const.tile([CIN, F], F32R)
            for half in range(2):
                pt = psA.tile([F, F], F32, tag="s2", name="pt")
                nc.tensor.matmul(
                    pt[:], pac_stage[:, half * 64:(half + 1) * 64], ident[:],
                    start=True, stop=True,
                )
                nc.scalar.activation(pac_wT[half * 64:(half + 1) * 64, :], pt[:], COPY)

            # Wp[t18 = ky*3+n][(px, c), o] = conv_w[o, c, ky, 2n+px]
            wp = const.tile([CIN, 18 * F], F32R)
            for ky in range(K):
                for n in range(3):
                    t18 = ky * 3 + n
                    for px in range(2):
                        pt = psA.tile([F, F], F32, tag="s2", name="pt")
                        # lhsT: [o(64 part), c(64)] strided pick of tap (ky, 2n+px)
                        lhsT = w_stage[:, ky * K + 2 * n + px::K * K]
                        nc.tensor.matmul(pt[:], lhsT, ident[:], start=True, stop=True)
                        nc.scalar.activation(
                            wp[px * 64:(px + 1) * 64, t18 * F:(t18 + 1) * F], pt[:], COPY
                        )

            # ---------------- phase tensors + halo zeroing ----------------
            zrow = const.tile([CIN, 2 * PC], F32)
            nc.gpsimd.memset(zrow[:], 0.0)

            hx = [
                hxp.tile([CIN, PR * PC], F32R, tag=f"hx{b}", name=f"hx{b}")
                for b in range(B_PER_CORE)
            ]
            for b in range(B_PER_CORE):
                t = hx[b]
                # top rows 0..1, bottom rows 130..131
                nc.scalar.activation(t[:, 0:2 * PC], zrow[:], COPY)
                nc.scalar.activation(t[:, 130 * PC:132 * PC], zrow[:], COPY)
                # left col 0 and right col 65 stripes (132 rows each)
                nc.scalar.activation(t[:, 0:PR * PC:PC], zrow[:, 0:PR], COPY)
                nc.scalar.activation(t[:, 65:PR * PC:PC], zrow[:, 0:PR], COPY)

            # ---------------- main pipeline ----------------
            for b in [bb for _ in range(repeat) for bb in range(B_PER_CORE)]:
                hxb = hx[b].rearrange("p (r c) -> p r c", c=PC)
                for t in range(8):  # 16-row x chunks
                    xt = xin.tile([CIN, 16 * W], F32R, tag="xt")
                    xtv = xt[:].rearrange("p (r c) -> p r c", r=16)
                    nc.sync.dma_start(
                        out=xtv,
                        in_=x[b, :, 16 * t:16 * t + 16, :].bitcast(F32R),
                    )
                    for h3 in range(2):  # 8-row halves
                        for px in range(2):  # x parity
                            ps = psA.tile([F, 8, 64], F32, tag="s1")
                            nc.tensor.matmul(
                                ps[:],
                                pac_wT[:],
                                xtv[:, 8 * h3:8 * h3 + 8, px::2],
                                start=True, stop=True,
                            )
                            # fused bias+prelu, writing the phase-image layout
                            r0 = 16 * t + 8 * h3 + 2
                            nc.scalar.activation(
                                hxb[px * 64:(px + 1) * 64, r0:r0 + 8, 1:65],
                                ps[:], PRELU, bias=b1[:], scale=1.0, alpha=a1[:],
                            )
                    if t >= 1:
                        _stage2_block(nc, tc, psA, obp, hxb, wp, b2, a2, out, b, t - 1)
                _stage2_block(nc, tc, psA, obp, hxb, wp, b2, a2, out, b, 7)

    nc.compile()
    return nc


def _stage2_block(nc, tc, psA, obp, hxb, wp, b2, a2, out, b, ib):
    """18 accumulating taps -> prelu -> dma for output rows [8*ib, 8*ib+8)."""
    PRELU = mybir.ActivationFunctionType.Prelu
    ps = psA.tile([F, 8, 64], F32, tag="s2")
    for ky in range(K):
        for n in range(3):
            t18 = ky * 3 + n
            r0 = 16 * ib + ky
            rhs = hxb[:, r0:min(r0 + 16, PR):2, n:n + 64]
            nc.tensor.matmul(
                ps[:], wp[:, t18 * F:(t18 + 1) * F], rhs,
                start=(t18 == 0), stop=(t18 == 17),
            )
    ot = obp.tile([F, 8, 64], F32, tag="ot")
    nc.scalar.activation(ot[:], ps[:], PRELU, bias=b2[:], scale=1.0, alpha=a2[:])
    nc.sync.dma_start(out=out[b, :, 8 * ib:8 * ib + 8, :], in_=ot[:])


def _get_nc(repeat=1):
    key = f"nc{repeat}"
    if key not in _CACHE:
        _CACHE[key] = _build(repeat)
    return _CACHE[key]


def kernel(x, guide, pac_w, pac_b, alpha1, alpha2, conv_w, conv_b, **_unused):
    # guide is mathematically unused (adaptive kernel == exp(0) == 1)
    del guide
    x = np.ascontiguousarray(x, dtype=np.float32)
    shared = {
        "pac_w": np.ascontiguousarray(pac_w, dtype=np.float32).reshape(F, CIN),
        "pac_b": np.ascontiguousarray(pac_b, dtype=np.float32),
        "alpha1": np.ascontiguousarray(alpha1, dtype=np.float32),
        "conv_w": np.ascontiguousarray(conv_w, dtype=np.float32).reshape(F, F * K * K),
        "conv_b": np.ascontiguousarray(conv_b, dtype=np.float32),
        "alpha2": np.ascontiguousarray(alpha2, dtype=np.float32),
    }
    in_maps = [
        {"x": np.ascontiguousarray(x[i * B_PER_CORE:(i + 1) * B_PER_CORE]), **shared}
        for i in range(N_CORES)
    ]
    nc = _get_nc()
    res = run_bass_kernel_spmd(
        nc, in_maps, list(range(N_CORES)), trace=_CACHE.get("trace", False)
    )
    _CACHE["last_result"] = res
    return np.concatenate([r["out"] for r in res.results], axis=0)
